# revision 7
# baseline (speedup 1.0000x reference)
"""Trainium2 Bass kernel for the cross-attention gating layer.

Computes, for x1 = input1[0] [S, src], x2 = input2[0] [T, tgt]:
    Q = x1 @ w_q.T; K = x2 @ w_k.T; V = x2 @ w_v.T
    attn = softmax(Q @ K.T / sqrt(128), axis=1)
    out  = (attn @ V) @ w_output.T          # [S, 1] gate
    res  = x1 * (1 - out)                   # (x1 if sum(x2)==0)

Key algebraic simplification: (attn @ V) @ w_output.T == attn @ (V @ w_output.T),
so V collapses into a single vector projection vg = x2 @ (w_output @ w_v).T [T].
Per query row s:  gate[s] = sum_t e[t,s]*vg[t] / sum_t e[t,s],
with e[t,s] = exp(scores[s,t]/sqrt(128)).

Sharding: sequence-parallel over S across 8 NeuronCores. Each core projects
its own T-shard of K^T / vg, the shards are AllGathered on-chip, then each
core runs its S-shard of the attention in transposed orientation (scores^T),
which makes every matmul operand land in its natural layout (no big
transposes anywhere; softmax denominator comes from a ones-column in the
attnPV stationary operand).
"""

import sys

sys.path.insert(0, "/opt/trn_rl_repo")

import numpy as np
import ml_dtypes

BF16 = ml_dtypes.bfloat16

NCORES = 8
S = 8192
T = 8192
SRC = 768
DK = 128
SL = S // NCORES  # 1024 local query rows
TL = T // NCORES  # 1024 local key rows
FC = SRC // 128  # 6 feature chunks
NQT = SL // 128  # 8 local q tiles
NTT = T // 128  # 64 global t tiles
KT_ELEMS = DK * TL  # 131072
GIN_ELEMS = KT_ELEMS + TL  # + vg shard

_CACHE = {}


def _build_nc():
    import concourse.bacc as bacc
    import concourse.mybir as mybir
    import concourse.tile as tile
    from concourse import masks

    dt = mybir.dt
    fp32 = dt.float32
    bf16 = dt.bfloat16

    nc = bacc.Bacc("TRN2", target_bir_lowering=False, debug=False, num_devices=NCORES)

    # I/O (all host-side pre-swizzled to partition-major contiguous layouts)
    x1f = nc.dram_tensor("x1f", [128, NQT, SRC], fp32, kind="ExternalInput").ap()
    x1t = nc.dram_tensor("x1t", [128, FC, SL], bf16, kind="ExternalInput").ap()
    x2t = nc.dram_tensor("x2t", [128, FC, TL], bf16, kind="ExternalInput").ap()
    wq = nc.dram_tensor("wq", [128, FC, DK], bf16, kind="ExternalInput").ap()
    wk = nc.dram_tensor("wk", [128, FC, DK], bf16, kind="ExternalInput").ap()
    wvo = nc.dram_tensor("wvo", [128, FC], bf16, kind="ExternalInput").ap()
    outp = nc.dram_tensor("outp", [128, NQT, SRC], fp32, kind="ExternalOutput").ap()

    gin = nc.dram_tensor("gin", [GIN_ELEMS], bf16).ap()
    gout = nc.dram_tensor("gout", [NCORES, GIN_ELEMS], bf16, addr_space="Shared").ap()
    vgall = nc.dram_tensor("vgall", [T], bf16).ap()
    din = nc.dram_tensor("din", [8], bf16).ap()
    dout = nc.dram_tensor("dout", [NCORES, 8], bf16, addr_space="Shared").ap()

    ISCALE = float(1.0 / np.sqrt(np.float32(128.0)))

    with tile.TileContext(nc) as tc:
        with (
            tc.tile_pool(name="const", bufs=1) as const,
            tc.tile_pool(name="work", bufs=1) as work,
            tc.tile_pool(name="pt", bufs=3) as ptp,
        ):
            # ---- dummy collective: absorbs cross-core launch stagger and
            # ncfw init while the prologue + projections run ---------------
            dtile = work.tile([1, 8], bf16)
            nc.vector.memset(dtile[:], 0.0)
            nc.sync.dma_start(din.rearrange("(o j) -> o j", o=1), dtile[:])
            nc.gpsimd.collective_compute(
                "AllGather",
                mybir.AluOpType.bypass,
                replica_groups=[list(range(NCORES))],
                ins=[din[:]],
                outs=[dout[:]],
            )

            # ---- loads (K-side first: they feed the real collective) ----
            wk_s = const.tile([128, FC * DK], bf16)
            nc.sync.dma_start(wk_s[:], wk.rearrange("p c m -> p (c m)"))
            wvo_s = const.tile([128, FC], bf16)
            nc.sync.dma_start(wvo_s[:], wvo[:])
            xt2 = const.tile([128, FC * TL], bf16)
            nc.sync.dma_start(xt2[:], x2t.rearrange("p c j -> p (c j)"))
            wq_s = const.tile([128, FC * DK], bf16)
            nc.sync.dma_start(wq_s[:], wq.rearrange("p c m -> p (c m)"))
            xt1 = const.tile([128, FC * SL], bf16)
            nc.sync.dma_start(xt1[:], x1t.rearrange("p c j -> p (c j)"))
            x1f_s = const.tile([128, NQT * SRC], fp32)
            nc.sync.dma_start(x1f_s[:], x1f.rearrange("p q f -> p (q f)"))

            ident = const.tile([128, 128], fp32)
            masks.make_identity(nc, ident[:])

            # ---- projections (Q^T, K^T, vg^T) --------------------------
            ppsum_cm = tc.tile_pool(name="ppsum", bufs=1, space="PSUM")
            ppsum = ppsum_cm.__enter__()
            qps = ppsum.tile([128, SL], fp32)
            kps = ppsum.tile([128, TL], fp32)
            vgps = ppsum.tile([1, TL], fp32)
            for h in range(2):
                qh = slice(512 * h, 512 * h + 512)
                for c in range(FC):
                    xs = slice(TL * c + 512 * h, TL * c + 512 * h + 512)
                    nc.tensor.matmul(
                        kps[:, qh],
                        wk_s[:, DK * c : DK * (c + 1)],
                        xt2[:, xs],
                        start=(c == 0),
                        stop=(c == FC - 1),
                    )
                for c in range(FC):
                    xs = slice(TL * c + 512 * h, TL * c + 512 * h + 512)
                    nc.tensor.matmul(
                        vgps[:, qh],
                        wvo_s[:, c : c + 1],
                        xt2[:, xs],
                        start=(c == 0),
                        stop=(c == FC - 1),
                    )

            kt_s = work.tile([128, TL], bf16)
            nc.vector.tensor_copy(kt_s[:], kps[:])
            vg_row = work.tile([1, TL], bf16)
            nc.vector.tensor_copy(vg_row[:], vgps[:])

            # ---- allgather K^T and vg (Q^T projection overlaps it) ------
            nc.sync.dma_start(
                gin[0:KT_ELEMS].rearrange("(p j) -> p j", p=128), kt_s[:]
            )
            nc.sync.dma_start(
                gin[KT_ELEMS:GIN_ELEMS].rearrange("(o j) -> o j", o=1), vg_row[:]
            )
            nc.gpsimd.collective_compute(
                "AllGather",
                mybir.AluOpType.bypass,
                replica_groups=[list(range(NCORES))],
                ins=[gin[:]],
                outs=[gout[:]],
            )

            for h in range(2):
                qh = slice(512 * h, 512 * h + 512)
                for c in range(FC):
                    xs = slice(SL * c + 512 * h, SL * c + 512 * h + 512)
                    nc.tensor.matmul(
                        qps[:, qh],
                        wq_s[:, DK * c : DK * (c + 1)],
                        xt1[:, xs],
                        start=(c == 0),
                        stop=(c == FC - 1),
                    )
            qT = const.tile([128, SL], bf16)
            nc.vector.tensor_copy(qT[:], qps[:])
            ppsum_cm.__exit__(None, None, None)

            ktAll = const.tile([128, T], bf16)
            nc.sync.dma_start(
                ktAll[:].rearrange("p (c j) -> p c j", c=NCORES),
                gout[:, 0:KT_ELEMS].rearrange("c (p j) -> p c j", p=128),
            )
            # vg columns: compact the gathered vg, then [64, 128] -> T -> [128, 64]
            nc.sync.dma_start(
                vgall.rearrange("(c j) -> c j", c=NCORES),
                gout[:, KT_ELEMS:GIN_ELEMS],
            )
            vgc = work.tile([128, NTT], bf16)
            nc.sync.dma_start(
                vgc[:],
                vgall.rearrange("(r p) -> r p", p=128),
                transpose=True,
            )
            # interleave vg columns with ones: vgi[:, 2t] = vg col t, vgi[:, 2t+1] = 1
            vgi = const.tile([128, 2 * NTT], bf16)
            nc.vector.memset(vgi[:], 1.0)
            nc.vector.tensor_copy(vgi[:, 0 : 2 * NTT : 2], vgc[:])

            # ---- main attention loop over t tiles ----------------------
            scps_cm = tc.tile_pool(name="scps", bufs=2, space="PSUM")
            scps = scps_cm.__enter__()
            avpsp_cm = tc.tile_pool(name="avps", bufs=1, space="PSUM")
            avpsp = avpsp_cm.__enter__()
            avps = avpsp.tile([64, SL], fp32)
            for tp in range(NTT // 2):
                pts = []
                for g in range(2):
                    tt = 2 * tp + g
                    sps = scps.tile([128, SL], fp32)
                    for h in range(2):
                        qh = slice(512 * h, 512 * h + 512)
                        nc.tensor.matmul(
                            sps[:, qh],
                            ktAll[:, 128 * tt : 128 * (tt + 1)],
                            qT[:, qh],
                            start=True,
                            stop=True,
                        )
                    pT = ptp.tile([128, SL], bf16)
                    nc.scalar.activation(
                        pT[:], sps[:], mybir.ActivationFunctionType.Exp, scale=ISCALE
                    )
                    pts.append(pT)
                # the two accumulating matmuls of a pair land on disjoint PE
                # column groups (tile_position) and run concurrently
                for h in range(2):
                    qh = slice(512 * h, 512 * h + 512)
                    for g in range(2):
                        tt = 2 * tp + g
                        nc.tensor.matmul(
                            avps[32 * g : 32 * g + 2, qh],
                            vgi[:, 2 * tt : 2 * tt + 2],
                            pts[g][:, qh],
                            start=(tp == 0),
                            stop=(tp == NTT // 2 - 1),
                            tile_position=(0, 32 * g),
                        )

            # ---- gate + output -----------------------------------------
            go_s = work.tile([2, SL], fp32)
            nc.vector.tensor_copy(go_s[:], avps[0:2, :])
            nc.vector.tensor_tensor(
                go_s[:], go_s[:], avps[32:34, :], mybir.AluOpType.add
            )
            avpsp_cm.__exit__(None, None, None)
            scps_cm.__exit__(None, None, None)

            trps_cm = tc.tile_pool(name="trps", bufs=2, space="PSUM")
            trps = trps_cm.__enter__()
            gq = work.tile([128, 2 * NQT], fp32)
            for q in range(NQT):
                trp = trps.tile([128, 2], fp32)
                nc.tensor.transpose(
                    trp[:], go_s[:, 128 * q : 128 * (q + 1)], ident[0:2, 0:2]
                )
                nc.vector.tensor_copy(gq[:, 2 * q : 2 * q + 2], trp[:])

            trps_cm.__exit__(None, None, None)

            recip = work.tile([128, NQT], fp32)
            nc.vector.reciprocal(recip[:], gq[:, 1 : 2 * NQT : 2])
            onem = work.tile([128, NQT], fp32)
            nc.vector.tensor_tensor(
                onem[:], gq[:, 0 : 2 * NQT : 2], recip[:], mybir.AluOpType.mult
            )
            nc.vector.tensor_scalar(
                onem[:], onem[:], -1.0, 1.0, mybir.AluOpType.mult, mybir.AluOpType.add
            )

            og = work.tile([128, NQT * SRC], fp32)
            for q in range(NQT):
                nc.vector.tensor_scalar(
                    og[:, SRC * q : SRC * (q + 1)],
                    x1f_s[:, SRC * q : SRC * (q + 1)],
                    onem[:, q : q + 1],
                    None,
                    mybir.AluOpType.mult,
                )
            nc.sync.dma_start(outp.rearrange("p q f -> p (q f)"), og[:])

    nc.compile()
    return nc


def _get_nc():
    if "nc" not in _CACHE:
        _CACHE["nc"] = _build_nc()
    return _CACHE["nc"]


def _prep_core_inputs(x1_sh, x2_sh, wq_sw, wk_sw, wvo_sw):
    # partition-major swizzles so every DMA is contiguous per partition
    x1f = np.ascontiguousarray(x1_sh.reshape(NQT, 128, SRC).transpose(1, 0, 2))
    x1t = np.ascontiguousarray(
        x1_sh.astype(BF16).T.reshape(FC, 128, SL).transpose(1, 0, 2)
    )
    x2t = np.ascontiguousarray(
        x2_sh.astype(BF16).T.reshape(FC, 128, TL).transpose(1, 0, 2)
    )
    return {
        "x1f": x1f,
        "x1t": x1t,
        "x2t": x2t,
        "wq": wq_sw,
        "wk": wk_sw,
        "wvo": wvo_sw,
    }


def kernel(input1, input2, w_q, w_k, w_v, w_output):
    from concourse.bass_utils import run_bass_kernel_spmd

    x1 = np.asarray(input1, dtype=np.float32)[0]
    x2 = np.asarray(input2, dtype=np.float32)[0]
    w_q = np.asarray(w_q, dtype=np.float32)
    w_k = np.asarray(w_k, dtype=np.float32)
    w_v = np.asarray(w_v, dtype=np.float32)
    w_output = np.asarray(w_output, dtype=np.float32)

    # Early-exit branch of the reference module.
    if x2.sum() == 0:
        return x1.copy()

    wq_sw = np.ascontiguousarray(
        w_q.T.astype(BF16).reshape(FC, 128, DK).transpose(1, 0, 2)
    )
    wk_sw = np.ascontiguousarray(
        w_k.T.astype(BF16).reshape(FC, 128, DK).transpose(1, 0, 2)
    )
    wvo = (w_output @ w_v)[0]  # [768]
    wvo_sw = np.ascontiguousarray(wvo.astype(BF16).reshape(FC, 128).T)

    in_maps = []
    for i in range(NCORES):
        sl = slice(i * SL, (i + 1) * SL)
        in_maps.append(
            _prep_core_inputs(x1[sl], x2[sl], wq_sw, wk_sw, wvo_sw)
        )

    nc = _get_nc()
    res = run_bass_kernel_spmd(nc, in_maps, list(range(NCORES)))

    out = np.empty((S, SRC), dtype=np.float32)
    for i in range(NCORES):
        o = res.results[i]["outp"]  # [128, NQT, SRC]
        out[i * SL : (i + 1) * SL] = o.transpose(1, 0, 2).reshape(SL, SRC)
    return out
